# revision 6
# baseline (speedup 1.0000x reference)
"""ALiBi causal attention on 8 Trainium2 NeuronCores.

Sharding: tensor-parallel over heads (2 heads/core). Core c owns global
heads c (steep slope) and 8+c (shallow slope) so that ALiBi block-skipping
gives every core the same instruction stream: steep heads only attend to
the ~5 k-blocks nearest the diagonal (older blocks decay below e^-8 of
the max weight), shallow heads attend to everything.

Output redistribution is pipelined: core c owns, for EVERY q-tile, the
64-query slice [512*qt + 64c, 512*qt + 64c + 64) of each batch. Each
attention group (b, qt) therefore feeds an independent 128KB AllToAll
piece that fires as soon as that group's output is normalized, and the
output projection for a 128-query block starts once its two pieces have
landed -- only the last piece + 16 matmuls remain after the final
attention group instead of a full-batch AllToAll + projection.

Layout choices (all chosen to avoid on-chip transposes):
  - x is passed host-transposed as xT [D=1024, B*T=4096] in bf16.
  - Q/K are produced in "head-transposed" layout [head_dim, tokens] with
    THREE augmented contraction rows carrying the ALiBi bias through the
    score matmul exactly:
      row 64: K=1,      Q=-8*slope*bf16(i)   (per-query term; any rounding
              cancels in softmax, so bf16 is safe)
      row 65: K=kb,     Q=C   where C = bf16(1024*slope); kb<=15 is exact
              in bf16 so C*kb accumulates exactly in f32
      row 66: K=kb,     Q=Dr  where Dr = bf16(1024*slope - C) mops up the
              rounding of C (double-bf16 trick)
    leaving only slope*p (p = partition index, exact f32) for the ScalarE
    exp bias -- which is then the same for every k-block of a head, so
    one exp instruction spans a PAIR of k-blocks (halves ACT op count;
    ACT exp throughput is the attention-phase bottleneck).
  - Scores are computed transposed: ST[k, q] = K'.T-block @ Q', so the
    softmax reduction (over k) aligns with the AV matmul contraction and
    the denominator falls out of a ones-column appended to V.
  - V is projected with Wv as the PE weights (8 long 512-col streams per
    chunk instead of 32 short 128-col ones) and moved into [token, dim]
    layout by XBAR transpose DMAs ([64,128] tiles) on the idle DMA
    engines -- the PE's weight-load port was the V-phase bottleneck.
  - Causal masking: only the diagonal-intersecting k-block per q-tile
    needs a 128x128 triangular min-clamp; fully-masked columns are never
    computed or streamed.

Tiles are deliberately small/chunked (xT per [k,512-token] block, Q/K per
[head, 512-token] chunk, V per [b, 512-token chunk]) because Tile's
dependency tracking is per-tile: projection chunk i feeds attention group
i 1:1 in the schedule. DMA queues: weights + xT + V-transposes + a2a-in
on the sync queue, a2a-out receives on the scalar queue placed after the
consuming-side exps they can never stall, constants/aug rows on the
GpSimd queue.
"""

import sys

if "/opt/trn_rl_repo" not in sys.path:
    sys.path.insert(0, "/opt/trn_rl_repo")

import numpy as np
import ml_dtypes

import concourse.bass as bass
import concourse.bacc as bacc
import concourse.tile as tile
import concourse.mybir as mybir
from concourse import bass_utils

BF16 = mybir.dt.bfloat16
F32 = mybir.dt.float32
NPBF16 = ml_dtypes.bfloat16

B, T, D = 2, 2048, 1024
H, HD = 16, 64
NC = 8
HPC = H // NC          # heads per core = 2
TOK = B * T            # 4096
TPC = TOK // NC        # tokens per core after a2a = 512 (256 per batch)
NKB = T // 128         # 16 k-blocks per sequence
NQT = T // 512         # 4 q-tiles per sequence
KAUG = HD + 3          # 67: head_dim + 3 aug rows
WINA = (4, 5, 5, 5)    # steep-head (slot A) kept k-blocks per q-tile

_COMPILED = None


def _kept(hl, qt):
    """k-blocks computed for head-slot hl in q-tile qt (contiguous,
    ending at the diagonal block 4*qt+3; may be odd-length for slot A)."""
    hi = 4 * qt + 4
    lo = max(0, hi - WINA[qt]) if hl == 0 else 0
    return list(range(lo, hi))


def _build():
    nc = bacc.Bacc("TRN2", target_bir_lowering=False, debug=False, num_devices=NC)

    xT_d = nc.dram_tensor("xT", [D, TOK], BF16, kind="ExternalInput")
    wqkv_d = nc.dram_tensor("wqkv", [D, 384], BF16, kind="ExternalInput")
    wo_d = nc.dram_tensor("wo", [D, D], BF16, kind="ExternalInput")
    qaug_d = nc.dram_tensor("qaug", [HPC * 3, T], BF16, kind="ExternalInput")
    kaug_d = nc.dram_tensor("kaug", [3, T], BF16, kind="ExternalInput")
    kbias_d = nc.dram_tensor("kbias", [128, HPC], F32, kind="ExternalInput")
    cap_d = nc.dram_tensor("cap", [128, 128], F32, kind="ExternalInput")
    out_d = nc.dram_tensor("out", [TPC, D], F32, kind="ExternalOutput")
    # a2a piece (b, qt): block p rows [128p,128p+128) = [128 dims of core
    # p's 2 heads, 64 queries 512qt+64p .. +64) -- contiguous per piece.
    ccinp = [
        [nc.dram_tensor(f"ccin{b}_{qt}", [NC * 128, 64], BF16, kind="Internal")
         for qt in range(NQT)]
        for b in range(B)
    ]
    ccoutp = [
        [nc.dram_tensor(f"ccout{b}_{qt}", [NC * 128, 64], BF16, kind="Internal")
         for qt in range(NQT)]
        for b in range(B)
    ]

    with tile.TileContext(nc) as tc:
        with (
            tc.tile_pool(name="const", bufs=1) as cpool,
            tc.tile_pool(name="work", bufs=1) as wpool,
            tc.tile_pool(name="pspair", bufs=2, space="PSUM") as pspair,
            tc.tile_pool(name="psot", bufs=2, space="PSUM") as psot,
            tc.tile_pool(name="ps", bufs=2, space="PSUM") as ps,
        ):
            # ---- sync queue: projection weights first, then xT chunks.
            # one merged weight tile: chunk k occupies cols [384k, 384k+384)
            # as [wq_k | wk_k | wv_k]
            wqkv_t = cpool.tile([128, 8 * 384], BF16, name="wqkv_t", tag="wqkv_t")
            for k in range(8):
                nc.sync.dma_start(
                    wqkv_t[:, 384 * k : 384 * (k + 1)], wqkv_d[128 * k : 128 * (k + 1), :]
                )
            xt = [[None] * (TOK // 512) for _ in range(8)]
            for tc8 in range(TOK // 512):
                for k in range(8):
                    t_ = cpool.tile([128, 512], BF16, name=f"xt{k}_{tc8}", tag=f"xt{k}_{tc8}")
                    nc.sync.dma_start(t_[:], xT_d[128 * k : 128 * (k + 1), 512 * tc8 : 512 * (tc8 + 1)])
                    xt[k][tc8] = t_

            # ---- PE warm-up: dependency-free matmuls on scratch data so
            # the HAM clock gate reaches 8/8 before the real work arrives.
            warm_in = cpool.tile([128, 512], BF16, name="warm_in", tag="warm_in")
            nc.vector.memset(warm_in[:], 0.0)
            for _ in range(30):
                wps = psot.tile([128, 512], F32, name="wps", tag="otv")
                nc.tensor.matmul(wps[:], warm_in[:, 0:128], warm_in[:], start=True, stop=True)

            # ---- collective warm-up: a tiny AllToAll absorbs the
            # first-collective setup cost while the PE does projections.
            ccw_in = nc.dram_tensor("ccwin", [128, 16], BF16, kind="Internal")
            ccw_out = nc.dram_tensor("ccwout", [128, 16], BF16, kind="Internal")
            nc.gpsimd.dma_start(ccw_in[:], kaug_d[0:1, 0:2048].rearrange("a (p c) -> (a p) c", p=128))
            nc.gpsimd.collective_compute(
                "AllToAll",
                mybir.AluOpType.bypass,
                replica_groups=[list(range(NC))],
                ins=[ccw_in[:]],
                outs=[ccw_out[:]],
            )

            # ---- constants + aug rows on the (otherwise idle) GpSimd
            # queue, in consumption order.
            kbias_t = cpool.tile([128, HPC], F32, name="kbias_t", tag="kbias_t")
            nc.gpsimd.dma_start(kbias_t[:], kbias_d[:])
            cap_t = cpool.tile([128, 128], F32, name="cap_t", tag="cap_t")
            nc.gpsimd.dma_start(cap_t[:], cap_d[:])

            qta = [[[None] * NQT for _ in range(HPC)] for _ in range(B)]
            kta = [[[None] * NQT for _ in range(HPC)] for _ in range(B)]
            for b in range(B):
                for hl in range(HPC):
                    for c in range(NQT):
                        q_ = cpool.tile([KAUG, 512], BF16, name=f"qta{b}{hl}{c}", tag=f"qta{b}{hl}{c}")
                        k_ = cpool.tile([KAUG, 512], BF16, name=f"kta{b}{hl}{c}", tag=f"kta{b}{hl}{c}")
                        qta[b][hl][c] = q_
                        kta[b][hl][c] = k_

            for b in range(B):
                for c in range(NQT):
                    for hl in range(HPC):
                        nc.gpsimd.dma_start(
                            qta[b][hl][c][64:67, :],
                            qaug_d[3 * hl : 3 * hl + 3, 512 * c : 512 * (c + 1)],
                        )
                        nc.gpsimd.dma_start(
                            kta[b][hl][c][64:67, :],
                            kaug_d[0:3, 512 * c : 512 * (c + 1)],
                        )
            # V: per (b, k-block): [128, 130]: 64 cols head A, ones col,
            # 64 cols head B, ones col.
            vt = [[None] * NKB for _ in range(B)]
            for b in range(B):
                for kb in range(NKB):
                    v_ = cpool.tile([128, 130], BF16, name=f"v{b}_{kb}", tag=f"v{b}_{kb}")
                    nc.vector.memset(v_.rearrange("p (a c) -> p a c", c=65)[:, :, 64], 1.0)
                    vt[b][kb] = v_

            # ---- phase 1: QKV projections (chunk-interleaved) ---------
            def qkv_chunk(tc8):
                b, cq = tc8 // NQT, tc8 % NQT
                for woff, dsts, eng in ((0, qta, "s"), (128, kta, "v")):
                    pp = ps.tile([128, 512], F32, name="pp", tag="mm512")
                    for k in range(8):
                        nc.tensor.matmul(
                            pp[:],
                            wqkv_t[:, 384 * k + woff : 384 * k + woff + 128],
                            xt[k][tc8][:],
                            start=(k == 0),
                            stop=(k == 7),
                        )
                    if eng == "s":
                        nc.scalar.copy(dsts[b][0][cq][0:64, :], pp[0:64, :])
                        nc.scalar.copy(dsts[b][1][cq][0:64, :], pp[64:128, :])
                    else:
                        nc.vector.tensor_copy(dsts[b][0][cq][0:64, :], pp[0:64, :])
                        nc.vector.tensor_copy(dsts[b][1][cq][0:64, :], pp[64:128, :])
                for j in range(4):
                    kb = 4 * cq + j
                    pv = ps.tile([128, 128], F32, name="pv", tag="mm512")
                    for k in range(8):
                        nc.tensor.matmul(
                            pv[:],
                            xt[k][tc8][:, 128 * j : 128 * (j + 1)],
                            wqkv_t[:, 384 * k + 256 : 384 * k + 384],
                            start=(k == 0),
                            stop=(k == 7),
                        )
                    nc.vector.tensor_copy(vt[b][kb][:, 0:64], pv[:, 0:64])
                    nc.vector.tensor_copy(vt[b][kb][:, 65:129], pv[:, 64:128])

            # ---- phase 2: attention for one (b, q-tile) ---------------
            def attn_group(b, qt):
                ots = []
                for hl in range(HPC):
                    ot = psot.tile([65, 512], F32, name="ot", tag="otv")
                    ots.append(ot)
                    kept = _kept(hl, qt)
                    pairs = [tuple(kept[i : i + 2]) for i in range(0, len(kept), 2)]
                    # In fully-diagonal pairs, put the larger column offset in
                    # slot 0: the single exp over [offs[0]:1024] then covers
                    # fewer dead columns. (Never the first pair, so the
                    # position-based start flag still covers all columns.)
                    pairs = [
                        (p[1], p[0]) if pi > 0 and len(p) == 2 and p[0] >= 4 * qt else p
                        for pi, p in enumerate(pairs)
                    ]
                    pend = []
                    for pi in range(len(pairs) + 1):
                        if pi < len(pairs):
                            pr_ = pairs[pi]
                            offs = [max(0, 128 * (kb - 4 * qt)) for kb in pr_]
                            pr = pspair.tile([128, 1024], F32, name="pr", tag="pair")
                            for s, (kb, off) in enumerate(zip(pr_, offs)):
                                nc.tensor.matmul(
                                    pr[:, 512 * s + off : 512 * (s + 1)],
                                    kta[b][hl][kb // 4][:, 128 * (kb % 4) : 128 * (kb % 4 + 1)],
                                    qta[b][hl][qt][:, off:512],
                                    start=True,
                                    stop=True,
                                )
                                if kb >= 4 * qt:
                                    nc.vector.tensor_tensor(
                                        pr[:, 512 * s + off : 512 * s + off + 128],
                                        pr[:, 512 * s + off : 512 * s + off + 128],
                                        cap_t[:],
                                        mybir.AluOpType.min,
                                    )
                            ex = wpool.tile([128, 1024], BF16, name="ex", tag="ex", bufs=6)
                            nc.scalar.activation(
                                ex[:, offs[0] : 512 * len(pr_)],
                                pr[:, offs[0] : 512 * len(pr_)],
                                mybir.ActivationFunctionType.Exp,
                                bias=kbias_t[:, hl : hl + 1],
                                scale=0.125,
                            )
                            pend.append((pr_, offs, ex))
                        if pi >= 1:
                            pj = pi - 1
                            pr_, offs, ex = pend.pop(0)
                            for s, (kb, off) in enumerate(zip(pr_, offs)):
                                nc.tensor.matmul(
                                    ot[:, off:512],
                                    vt[b][kb][:, 65 * hl : 65 * hl + 65],
                                    ex[:, 512 * s + off : 512 * (s + 1)],
                                    start=(pj == 0 and s == 0),
                                    stop=(pj == len(pairs) - 1 and s == len(pr_) - 1),
                                )
                # Copy OT out of PSUM immediately (one op per head, split
                # across ScalarE/VectorE) so the psot slots release for the
                # next group; the whole normalize chain then runs from SBUF
                # off the inter-group critical path. The very last group
                # instead normalizes straight from PSUM (nothing competes
                # for its psot slots) to shorten the path to the final
                # collective's doorbell.
                lean = b == B - 1 and qt == NQT - 1
                den2 = wpool.tile([1, 1024], F32, name="den2", tag="den2", bufs=2)
                if lean:
                    nc.vector.tensor_copy(den2[:, 0:512], ots[0][64:65, :])
                    nc.vector.tensor_copy(den2[:, 512:1024], ots[1][64:65, :])
                else:
                    otf0 = wpool.tile([65, 512], F32, name="otf0", tag="otf0", bufs=3)
                    otf1 = wpool.tile([128, 512], F32, name="otf1", tag="otf1", bufs=3)
                    denb = wpool.tile([1, 512], F32, name="denb", tag="denb", bufs=2)
                    nc.scalar.copy(otf0[:], ots[0][:])
                    nc.vector.tensor_copy(otf1[64:128, :], ots[1][0:64, :])
                    nc.vector.tensor_copy(denb[:], ots[1][64:65, :])
                    nc.vector.tensor_copy(den2[:, 0:512], otf0[64:65, :])
                    nc.vector.tensor_copy(den2[:, 512:1024], denb[:])
                # reciprocal on the [1,1024] row FIRST (cheap), then
                # broadcast the reciprocal.
                den2i = wpool.tile([1, 1024], F32, name="den2i", tag="den2i", bufs=2)
                nc.vector.reciprocal_approx_fast(den2i[:], den2[:])
                bci = wpool.tile([128, 1024], F32, name="bci", tag="bci", bufs=2)
                nc.gpsimd.partition_broadcast(bci[:], den2i[:])
                otn = wpool.tile([128, 512], BF16, name="otn", tag="otn", bufs=4)
                if lean:
                    nc.vector.tensor_tensor(
                        otn[0:64, :], ots[0][0:64, :], bci[0:64, 0:512], mybir.AluOpType.mult
                    )
                    nc.vector.tensor_tensor(
                        otn[64:128, :], ots[1][0:64, :], bci[64:128, 512:1024], mybir.AluOpType.mult
                    )
                else:
                    nc.vector.tensor_tensor(
                        otn[0:64, :], otf0[0:64, :], bci[0:64, 0:512], mybir.AluOpType.mult
                    )
                    nc.vector.tensor_tensor(
                        otn[64:128, :], otf1[64:128, :], bci[64:128, 512:1024], mybir.AluOpType.mult
                    )
                # scatter the 8 per-core 64-query slices into this piece's
                # a2a input (single DMA, iteration order (dim-row, p, q)).
                nc.sync.dma_start(
                    ccinp[b][qt].rearrange("(p r) q -> r p q", p=8),
                    otn.rearrange("r (p q) -> r p q", p=8),
                )

            def cc_piece(b, qt):
                nc.gpsimd.collective_compute(
                    "AllToAll",
                    mybir.AluOpType.bypass,
                    replica_groups=[list(range(NC))],
                    ins=[ccinp[b][qt][:]],
                    outs=[ccoutp[b][qt][:]],
                )

            # ---- phase 4: output projection ---------------------------
            # atb[b][tb] [128 dims, (k 8, h 2, q 64)]: my queries for
            # q-tiles {2tb, 2tb+1}, all 16 heads' dims (k = source core).
            atb = [[None] * 2 for _ in range(B)]
            for b in range(B):
                for tb in range(2):
                    a_ = cpool.tile([128, 1024], BF16, name=f"at{b}_{tb}", tag=f"at{b}_{tb}")
                    atb[b][tb] = a_

            def yrecv(b, qt):
                # scalar-queue trigger: placed in program order after exps
                # that are guaranteed past this piece's completion.
                nc.scalar.dma_start(
                    atb[b][qt // 2]
                    .rearrange("d (k h q) -> d k h q", k=8, h=2)[:, :, qt % 2, :],
                    ccoutp[b][qt].rearrange("(k d) q -> d k q", k=8),
                )

            def ypiece(b, tb, n):
                yp = ps.tile([128, 512], F32, name="yp", tag="mm512")
                for k in range(8):
                    nc.tensor.matmul(
                        yp[:],
                        atb[b][tb][:, 128 * k : 128 * (k + 1)],
                        wo_t[:, D * k + 512 * n : D * k + 512 * (n + 1)],
                        start=(k == 0),
                        stop=(k == 7),
                    )
                ys = wpool.tile([128, 512], F32, name="ys", tag="ys", bufs=2)
                nc.vector.tensor_copy(ys[:], yp[:])
                nc.sync.dma_start(
                    out_d[256 * b + 128 * tb : 256 * b + 128 * (tb + 1), 512 * n : 512 * (n + 1)],
                    ys[:],
                )

            # ---- schedule -------------------------------------------
            for qt in range(NQT):
                qkv_chunk(qt)
                attn_group(0, qt)
                cc_piece(0, qt)
            # wo arrives during batch-1 attention on the sync queue
            wo_t = cpool.tile([128, 8 * D], BF16, name="wo_t", tag="wo_t")
            for k in range(8):
                nc.sync.dma_start(wo_t[:, D * k : D * (k + 1)], wo_d[128 * k : 128 * (k + 1), :])

            qkv_chunk(NQT + 0)
            attn_group(1, 0)
            cc_piece(1, 0)
            yrecv(0, 0)
            yrecv(0, 1)

            qkv_chunk(NQT + 1)
            attn_group(1, 1)
            cc_piece(1, 1)
            yrecv(0, 2)
            yrecv(0, 3)
            ypiece(0, 0, 0)
            ypiece(0, 0, 1)

            qkv_chunk(NQT + 2)
            attn_group(1, 2)
            cc_piece(1, 2)
            ypiece(0, 1, 0)
            ypiece(0, 1, 1)

            qkv_chunk(NQT + 3)
            attn_group(1, 3)
            cc_piece(1, 3)
            yrecv(1, 0)
            yrecv(1, 1)
            yrecv(1, 2)
            yrecv(1, 3)
            ypiece(1, 0, 0)
            ypiece(1, 0, 1)
            ypiece(1, 1, 0)
            ypiece(1, 1, 1)

    nc.compile()
    return nc


def _host_inputs(x, Wq, Wk, Wv, Wo):
    x = np.asarray(x, dtype=np.float32)
    Wq, Wk, Wv, Wo = (np.asarray(w, dtype=np.float32) for w in (Wq, Wk, Wv, Wo))
    toks = x.reshape(TOK, D)
    xT = np.ascontiguousarray(toks.T).astype(NPBF16)
    base = 2.0 ** (-8.0 / H)

    cap = np.where(
        np.arange(128)[:, None] <= np.arange(128)[None, :], 3.0e38, -1.0e9
    ).astype(np.float32)
    pos = np.arange(T, dtype=np.float32)
    pos_bf = pos.astype(NPBF16).astype(np.float32)
    kbrow = np.floor(pos / 128.0).astype(NPBF16)  # k-block index, exact
    ones_row = np.ones(T, dtype=NPBF16)
    kaug = np.stack([ones_row, kbrow, kbrow])  # rows 64..66 of K'

    in_maps = []
    for c in range(NC):
        heads = [c, 8 + c]  # steep slot A, shallow slot B
        rows = np.concatenate([np.arange(64 * g, 64 * (g + 1)) for g in heads])
        qaug = np.zeros((HPC * 3, T), dtype=NPBF16)
        kbias = np.zeros((128, HPC), dtype=np.float32)
        for hl, g in enumerate(heads):
            slope = float(base ** (g + 1))
            qaug[3 * hl + 0] = (-8.0 * slope * pos_bf).astype(NPBF16)
            cc = NPBF16(1024.0 * slope)
            dr = NPBF16(1024.0 * slope - float(cc))
            qaug[3 * hl + 1] = cc
            qaug[3 * hl + 2] = dr
            kbias[:, hl] = slope * np.arange(128)
        in_maps.append(
            {
                "xT": xT,
                "wqkv": np.ascontiguousarray(
                    np.concatenate(
                        [Wq[rows, :].T, Wk[rows, :].T, Wv[rows, :].T], axis=1
                    )
                ).astype(NPBF16),
                "wo": None,  # filled below (same for all cores)
                "qaug": qaug,
                "kaug": kaug,
                "kbias": kbias,
                "cap": cap,
            }
        )
    # Wo rows permuted to match the concat order the a2a produces:
    # source core p contributes [head p dims ; head 8+p dims].
    perm = np.concatenate(
        [
            np.concatenate(
                [np.arange(64 * p, 64 * (p + 1)), np.arange(64 * (8 + p), 64 * (9 + p))]
            )
            for p in range(NC)
        ]
    )
    wo_t = np.ascontiguousarray(Wo.T[perm, :]).astype(NPBF16)
    for m in in_maps:
        m["wo"] = wo_t
    return in_maps


def get_compiled():
    global _COMPILED
    if _COMPILED is None:
        _COMPILED = _build()
    return _COMPILED


def run(x, Wq, Wk, Wv, Wo, trace=False, **trace_kwargs):
    nc = get_compiled()
    in_maps = _host_inputs(x, Wq, Wk, Wv, Wo)
    res = bass_utils.run_bass_kernel_spmd(
        nc, in_maps, core_ids=list(range(NC)), trace=trace, **trace_kwargs
    )
    full = np.empty((TOK, D), dtype=np.float32)
    # o row [256b + 128tb + 64h + j] = batch b, query 512*(2tb+h) + 64c + j
    for c in range(NC):
        o = res.results[c]["out"]
        for b in range(B):
            for qt in range(NQT):
                full[T * b + 512 * qt + 64 * c : T * b + 512 * qt + 64 * c + 64, :] = o[
                    256 * b + 64 * qt : 256 * b + 64 * qt + 64
                ]
    return full.reshape(B, T, D), res


def kernel(x, Wq, Wk, Wv, Wo):
    out, _ = run(x, Wq, Wk, Wv, Wo)
    return out


# revision 13
# speedup vs baseline: 1.0078x; 1.0078x over previous
"""ALiBi causal attention on 8 Trainium2 NeuronCores.

Sharding: tensor-parallel over heads (2 heads/core). Core c owns global
heads c (steep slope) and 8+c (shallow slope) so that ALiBi block-skipping
gives every core the same instruction stream: steep heads only attend to
the ~5 k-blocks nearest the diagonal (older blocks decay below e^-8 of
the max weight), shallow heads attend to everything.

Output redistribution is pipelined: core c owns, for EVERY q-tile, the
64-query slice [512*qt + 64c, 512*qt + 64c + 64) of each batch. Each
attention group (b, qt) therefore feeds an independent 128KB AllToAll
piece that fires as soon as that group's output is normalized, and the
output projection for a 128-query block starts once its two pieces have
landed -- only the last piece + 16 matmuls remain after the final
attention group instead of a full-batch AllToAll + projection.

Layout choices (all chosen to avoid on-chip transposes):
  - x is passed host-transposed as xT [D=1024, B*T=4096] in bf16.
  - Q/K are produced in "head-transposed" layout [head_dim, tokens] with
    THREE augmented contraction rows carrying the ALiBi bias through the
    score matmul exactly:
      row 64: K=1,      Q=-8*slope*bf16(i)   (per-query term; any rounding
              cancels in softmax, so bf16 is safe)
      row 65: K=kb,     Q=C   where C = bf16(1024*slope); kb<=15 is exact
              in bf16 so C*kb accumulates exactly in f32
      row 66: K=kb,     Q=Dr  where Dr = bf16(1024*slope - C) mops up the
              rounding of C (double-bf16 trick)
    leaving only slope*p (p = partition index, exact f32) for the ScalarE
    exp bias -- which is then the same for every k-block of a head, so
    one exp instruction spans a PAIR of k-blocks (halves ACT op count;
    ACT exp throughput is the attention-phase bottleneck).
  - Scores are computed transposed: ST[k, q] = K'.T-block @ Q', so the
    softmax reduction (over k) aligns with the AV matmul contraction and
    the denominator falls out of a ones-column appended to V.
  - V is projected with Wv as the PE weights (8 long 512-col streams per
    chunk instead of 32 short 128-col ones) and moved into [token, dim]
    layout by XBAR transpose DMAs ([64,128] tiles) on the idle DMA
    engines -- the PE's weight-load port was the V-phase bottleneck.
  - Causal masking: only the diagonal-intersecting k-block per q-tile
    needs a 128x128 triangular min-clamp; fully-masked columns are never
    computed or streamed.

Tiles are deliberately small/chunked (xT per [k,512-token] block, Q/K per
[head, 512-token] chunk, V per [b, 512-token chunk]) because Tile's
dependency tracking is per-tile: projection chunk i feeds attention group
i 1:1 in the schedule. DMA queues: weights + xT + V-transposes + a2a-in
on the sync queue, a2a-out receives on the scalar queue placed after the
consuming-side exps they can never stall, constants/aug rows on the
GpSimd queue.
"""

import sys

if "/opt/trn_rl_repo" not in sys.path:
    sys.path.insert(0, "/opt/trn_rl_repo")

import numpy as np
import ml_dtypes

import concourse.bass as bass
import concourse.bacc as bacc
import concourse.tile as tile
import concourse.mybir as mybir
from concourse import bass_utils

BF16 = mybir.dt.bfloat16
F32 = mybir.dt.float32
NPBF16 = ml_dtypes.bfloat16

B, T, D = 2, 2048, 1024
H, HD = 16, 64
NC = 8
HPC = H // NC          # heads per core = 2
TOK = B * T            # 4096
TPC = TOK // NC        # tokens per core after a2a = 512 (256 per batch)
NKB = T // 128         # 16 k-blocks per sequence
NQT = T // 512         # 4 q-tiles per sequence
KAUG = HD + 3          # 67: head_dim + 3 aug rows
WINA = (4, 5, 5, 5)    # steep-head (slot A) kept k-blocks per q-tile

_COMPILED = None


def _kept(hl, qt):
    """k-blocks computed for head-slot hl in q-tile qt (contiguous,
    ending at the diagonal block 4*qt+3; may be odd-length for slot A)."""
    hi = 4 * qt + 4
    lo = max(0, hi - WINA[qt]) if hl == 0 else 0
    return list(range(lo, hi))


def _build():
    nc = bacc.Bacc("TRN2", target_bir_lowering=False, debug=False, num_devices=NC)

    xT_d = nc.dram_tensor("xT", [D, TOK], BF16, kind="ExternalInput")
    wqkv_d = nc.dram_tensor("wqkv", [D, 384], BF16, kind="ExternalInput")
    wo_d = nc.dram_tensor("wo", [D, D], BF16, kind="ExternalInput")
    qaug_d = nc.dram_tensor("qaug", [HPC * 3, T], BF16, kind="ExternalInput")
    kaug_d = nc.dram_tensor("kaug", [3, T], BF16, kind="ExternalInput")
    kbias_d = nc.dram_tensor("kbias", [128, HPC], F32, kind="ExternalInput")
    cap_d = nc.dram_tensor("cap", [128, 128], F32, kind="ExternalInput")
    out_d = nc.dram_tensor("out", [TPC, D], F32, kind="ExternalOutput")
    # a2a pieces: block p rows [128p,128p+128) = [128 dims of core p's 2
    # heads, owned queries]. Batch 0 is not tail-critical: 2 pieces of a
    # q-tile PAIR each (cols = (qt&1, 64q)). Batch 1: 4 per-qt pieces so
    # only the last 128KB piece trails the final attention group.
    ccin0 = [nc.dram_tensor(f"ccin0_{tb}", [NC * 128, 128], BF16, kind="Internal")
             for tb in range(2)]
    ccout0 = [nc.dram_tensor(f"ccout0_{tb}", [NC * 128, 128], BF16, kind="Internal")
              for tb in range(2)]
    ccin1 = [nc.dram_tensor(f"ccin1_{qt}", [NC * 128, 64], BF16, kind="Internal")
             for qt in range(NQT)]
    ccout1 = [nc.dram_tensor(f"ccout1_{qt}", [NC * 128, 64], BF16, kind="Internal")
              for qt in range(NQT)]

    with tile.TileContext(nc) as tc:
        with (
            tc.tile_pool(name="const", bufs=1) as cpool,
            tc.tile_pool(name="work", bufs=1) as wpool,
            tc.tile_pool(name="pspair", bufs=2, space="PSUM") as pspair,
            tc.tile_pool(name="psot", bufs=2, space="PSUM") as psot,
            tc.tile_pool(name="ps", bufs=2, space="PSUM") as ps,
        ):
            # ---- sync queue: projection weights first, then xT chunks.
            # one merged weight tile: chunk k occupies cols [384k, 384k+384)
            # as [wq_k | wk_k | wv_k]
            wqkv_t = cpool.tile([128, 8 * 384], BF16, name="wqkv_t", tag="wqkv_t")
            for k in range(8):
                nc.sync.dma_start(
                    wqkv_t[:, 384 * k : 384 * (k + 1)], wqkv_d[128 * k : 128 * (k + 1), :]
                )
            xt = [[None] * (TOK // 512) for _ in range(8)]
            for tc8 in range(TOK // 512):
                for k in range(8):
                    t_ = cpool.tile([128, 512], BF16, name=f"xt{k}_{tc8}", tag=f"xt{k}_{tc8}")
                    nc.sync.dma_start(t_[:], xT_d[128 * k : 128 * (k + 1), 512 * tc8 : 512 * (tc8 + 1)])
                    xt[k][tc8] = t_

            # ---- PE warm-up: dependency-free matmuls on scratch data so
            # the HAM clock gate reaches 8/8 before the real work arrives.
            warm_in = cpool.tile([128, 512], BF16, name="warm_in", tag="warm_in")
            nc.vector.memset(warm_in[:], 0.0)
            for _ in range(30):
                wps = psot.tile([128, 512], F32, name="wps", tag="otv")
                nc.tensor.matmul(wps[:], warm_in[:, 0:128], warm_in[:], start=True, stop=True)

            # ---- collective warm-up: a tiny AllToAll absorbs the
            # first-collective setup cost while the PE does projections.
            ccw_in = nc.dram_tensor("ccwin", [128, 16], BF16, kind="Internal")
            ccw_out = nc.dram_tensor("ccwout", [128, 16], BF16, kind="Internal")
            nc.gpsimd.dma_start(ccw_in[:], kaug_d[0:1, 0:2048].rearrange("a (p c) -> (a p) c", p=128))
            nc.gpsimd.collective_compute(
                "AllToAll",
                mybir.AluOpType.bypass,
                replica_groups=[list(range(NC))],
                ins=[ccw_in[:]],
                outs=[ccw_out[:]],
            )

            # ---- constants + aug rows on the (otherwise idle) GpSimd
            # queue, in consumption order.
            kbias_t = cpool.tile([128, HPC], F32, name="kbias_t", tag="kbias_t")
            nc.gpsimd.dma_start(kbias_t[:], kbias_d[:])
            cap_t = cpool.tile([128, 128], F32, name="cap_t", tag="cap_t")
            nc.gpsimd.dma_start(cap_t[:], cap_d[:])

            qta = [[[None] * NQT for _ in range(HPC)] for _ in range(B)]
            kta = [[[None] * NQT for _ in range(HPC)] for _ in range(B)]
            for b in range(B):
                for hl in range(HPC):
                    for c in range(NQT):
                        q_ = cpool.tile([KAUG, 512], BF16, name=f"qta{b}{hl}{c}", tag=f"qta{b}{hl}{c}")
                        k_ = cpool.tile([KAUG, 512], BF16, name=f"kta{b}{hl}{c}", tag=f"kta{b}{hl}{c}")
                        qta[b][hl][c] = q_
                        kta[b][hl][c] = k_

            for b in range(B):
                for c in range(NQT):
                    for hl in range(HPC):
                        nc.gpsimd.dma_start(
                            qta[b][hl][c][64:67, :],
                            qaug_d[3 * hl : 3 * hl + 3, 512 * c : 512 * (c + 1)],
                        )
                        nc.gpsimd.dma_start(
                            kta[b][hl][c][64:67, :],
                            kaug_d[0:3, 512 * c : 512 * (c + 1)],
                        )
            # V: per (b, k-block): [128, 130]: ones col, 64 cols head A,
            # ones col, 64 cols head B -- ones FIRST so the softmax
            # denominator lands on partition 0 of the AV output (where
            # partition_broadcast can read it).
            vt = [[None] * NKB for _ in range(B)]
            for b in range(B):
                for kb in range(NKB):
                    v_ = cpool.tile([128, 130], BF16, name=f"v{b}_{kb}", tag=f"v{b}_{kb}")
                    nc.vector.memset(v_.rearrange("p (a c) -> p a c", c=65)[:, :, 0], 1.0)
                    vt[b][kb] = v_

            # ---- phase 1: QKV projections (chunk-interleaved) ---------
            def qkv_chunk(tc8):
                b, cq = tc8 // NQT, tc8 % NQT
                for woff, dsts, eng in ((0, qta, "s"), (128, kta, "v")):
                    pp = ps.tile([128, 512], F32, name="pp", tag="mm512")
                    for k in range(8):
                        nc.tensor.matmul(
                            pp[:],
                            wqkv_t[:, 384 * k + woff : 384 * k + woff + 128],
                            xt[k][tc8][:],
                            start=(k == 0),
                            stop=(k == 7),
                        )
                    if eng == "s":
                        nc.scalar.copy(dsts[b][0][cq][0:64, :], pp[0:64, :])
                        nc.scalar.copy(dsts[b][1][cq][0:64, :], pp[64:128, :])
                    else:
                        nc.vector.tensor_copy(dsts[b][0][cq][0:64, :], pp[0:64, :])
                        nc.vector.tensor_copy(dsts[b][1][cq][0:64, :], pp[64:128, :])
                for j in range(4):
                    kb = 4 * cq + j
                    pv = ps.tile([128, 128], F32, name="pv", tag="mm512")
                    for k in range(8):
                        nc.tensor.matmul(
                            pv[:],
                            xt[k][tc8][:, 128 * j : 128 * (j + 1)],
                            wqkv_t[:, 384 * k + 256 : 384 * k + 384],
                            start=(k == 0),
                            stop=(k == 7),
                        )
                    nc.vector.tensor_copy(vt[b][kb][:, 1:65], pv[:, 0:64])
                    nc.vector.tensor_copy(vt[b][kb][:, 66:130], pv[:, 64:128])

            # ---- phase 2: attention for one (b, q-tile) ---------------
            def attn_group(b, qt):
                ots = []
                for hl in range(HPC):
                    ot = psot.tile([65, 512], F32, name="ot", tag="otv")
                    ots.append(ot)
                    kept = _kept(hl, qt)
                    pairs = [tuple(kept[i : i + 2]) for i in range(0, len(kept), 2)]
                    # In fully-diagonal pairs, put the larger column offset in
                    # slot 0: the single exp over [offs[0]:1024] then covers
                    # fewer dead columns. (Never the first pair, so the
                    # position-based start flag still covers all columns.)
                    pairs = [
                        (p[1], p[0]) if pi > 0 and len(p) == 2 and p[0] >= 4 * qt else p
                        for pi, p in enumerate(pairs)
                    ]
                    pend = []
                    for pi in range(len(pairs) + 1):
                        if pi < len(pairs):
                            pr_ = pairs[pi]
                            offs = [max(0, 128 * (kb - 4 * qt)) for kb in pr_]
                            pr = pspair.tile([128, 1024], F32, name="pr", tag="pair")
                            for s, (kb, off) in enumerate(zip(pr_, offs)):
                                nc.tensor.matmul(
                                    pr[:, 512 * s + off : 512 * (s + 1)],
                                    kta[b][hl][kb // 4][:, 128 * (kb % 4) : 128 * (kb % 4 + 1)],
                                    qta[b][hl][qt][:, off:512],
                                    start=True,
                                    stop=True,
                                )
                                if kb >= 4 * qt:
                                    nc.vector.tensor_tensor(
                                        pr[:, 512 * s + off : 512 * s + off + 128],
                                        pr[:, 512 * s + off : 512 * s + off + 128],
                                        cap_t[:],
                                        mybir.AluOpType.min,
                                    )
                            ex = wpool.tile([128, 1024], BF16, name="ex", tag="ex", bufs=6)
                            nc.scalar.activation(
                                ex[:, offs[0] : 512 * len(pr_)],
                                pr[:, offs[0] : 512 * len(pr_)],
                                mybir.ActivationFunctionType.Exp,
                                bias=kbias_t[:, hl : hl + 1],
                                scale=0.125,
                            )
                            pend.append((pr_, offs, ex))
                        if pi >= 1:
                            pj = pi - 1
                            pr_, offs, ex = pend.pop(0)
                            for s, (kb, off) in enumerate(zip(pr_, offs)):
                                nc.tensor.matmul(
                                    ot[:, off:512],
                                    vt[b][kb][:, 65 * hl : 65 * hl + 65],
                                    ex[:, 512 * s + off : 512 * (s + 1)],
                                    start=(pj == 0 and s == 0),
                                    stop=(pj == len(pairs) - 1 and s == len(pr_) - 1),
                                )
                # Per-head normalize: head A's chain (copy -> den-broadcast
                # -> reciprocal -> multiply -> a2a-half DMA) runs while head
                # B's matmuls are still streaming, so only head B's short
                # chain sits between the last AV and the piece's doorbell.
                for hl in range(HPC):
                    otf = wpool.tile([65, 512], F32, name=f"otf{hl}", tag=f"otf{hl}", bufs=2)
                    nc.vector.tensor_copy(otf[:], ots[hl][:])
                    bch = wpool.tile([65, 512], F32, name=f"bch{hl}", tag=f"bch{hl}", bufs=2)
                    nc.gpsimd.partition_broadcast(bch[:], otf[0:1, :], channels=65)
                    rch = wpool.tile([65, 512], F32, name=f"rch{hl}", tag=f"rch{hl}", bufs=2)
                    nc.vector.reciprocal_approx_fast(rch[:], bch[:])
                    otn = wpool.tile([65, 512], BF16, name=f"otn{hl}", tag=f"otn{hl}", bufs=2)
                    nc.vector.tensor_tensor(
                        otn[:], otf[:], rch[:], mybir.AluOpType.mult
                    )
                    # scatter the 8 per-core 64-query slices of this head's
                    # half-rows into the piece (iteration (row, p, q)).
                    if b == 0:
                        dst = ccin0[qt // 2].rearrange(
                            "(p h r) (t q) -> h t r p q", p=8, h=2, t=2
                        )[hl][qt % 2]
                    else:
                        dst = ccin1[qt].rearrange("(p h r) q -> h r p q", p=8, h=2)[hl]
                    nc.sync.dma_start(dst, otn[1:65, :].rearrange("r (p q) -> r p q", p=8))

            def cc_piece(b, qt):
                cin = ccin0[qt // 2] if b == 0 else ccin1[qt]
                cout = ccout0[qt // 2] if b == 0 else ccout1[qt]
                nc.gpsimd.collective_compute(
                    "AllToAll",
                    mybir.AluOpType.bypass,
                    replica_groups=[list(range(NC))],
                    ins=[cin[:]],
                    outs=[cout[:]],
                )

            # ---- phase 4: output projection ---------------------------
            # atb[b][tb] [128 dims, (k 8, h 2, q 64)]: my queries for
            # q-tiles {2tb, 2tb+1}, all 16 heads' dims (k = source core).
            atb = [[None] * 2 for _ in range(B)]
            for b in range(B):
                for tb in range(2):
                    a_ = cpool.tile([128, 1024], BF16, name=f"at{b}_{tb}", tag=f"at{b}_{tb}")
                    atb[b][tb] = a_

            def yrecv0(tb):
                # scalar-queue trigger: placed in program order after exps
                # that are guaranteed past this piece's completion.
                nc.scalar.dma_start(
                    atb[0][tb].rearrange("d (k t q) -> d k t q", k=8, t=2),
                    ccout0[tb].rearrange("(k h d) (t q) -> (h d) k t q", k=8, h=2, t=2),
                )

            def yrecv1(qt):
                nc.scalar.dma_start(
                    atb[1][qt // 2]
                    .rearrange("d (k h q) -> d k h q", k=8, h=2)[:, :, qt % 2, :],
                    ccout1[qt].rearrange("(k d) q -> d k q", k=8),
                )

            def ypiece(b, tb, n):
                yp = ps.tile([128, 512], F32, name="yp", tag="mm512")
                for k in range(8):
                    nc.tensor.matmul(
                        yp[:],
                        atb[b][tb][:, 128 * k : 128 * (k + 1)],
                        wo_t[:, D * k + 512 * n : D * k + 512 * (n + 1)],
                        start=(k == 0),
                        stop=(k == 7),
                    )
                ys = wpool.tile([128, 512], F32, name="ys", tag="ys", bufs=2)
                nc.vector.tensor_copy(ys[:], yp[:])
                nc.sync.dma_start(
                    out_d[256 * b + 128 * tb : 256 * b + 128 * (tb + 1), 512 * n : 512 * (n + 1)],
                    ys[:],
                )

            def ypiece_wide(tb):
                # tail-only: 1024-col streams from the pair pool (idle by
                # now), single PSUM copy, out-DMA on the post-exp-idle
                # scalar queue.
                yp = pspair.tile([128, 1024], F32, name="ypw", tag="pair")
                for n in range(2):
                    for k in range(8):
                        nc.tensor.matmul(
                            yp[:, 512 * n : 512 * (n + 1)],
                            atb[1][tb][:, 128 * k : 128 * (k + 1)],
                            wo_t[:, D * k + 512 * n : D * k + 512 * (n + 1)],
                            start=(k == 0),
                            stop=(k == 7),
                        )
                ys = wpool.tile([128, 1024], F32, name="ysw", tag="ysw", bufs=2)
                nc.vector.tensor_copy(ys[:], yp[:])
                nc.scalar.dma_start(
                    out_d[256 + 128 * tb : 256 + 128 * (tb + 1), :], ys[:]
                )

            # ---- schedule -------------------------------------------
            for qt in range(NQT):
                qkv_chunk(qt)
                attn_group(0, qt)
                if qt % 2 == 1:
                    cc_piece(0, qt)
            # wo arrives during batch-1 attention on the sync queue
            wo_t = cpool.tile([128, 8 * D], BF16, name="wo_t", tag="wo_t")
            for k in range(8):
                nc.sync.dma_start(wo_t[:, D * k : D * (k + 1)], wo_d[128 * k : 128 * (k + 1), :])

            qkv_chunk(NQT + 0)
            attn_group(1, 0)
            cc_piece(1, 0)
            yrecv0(0)

            qkv_chunk(NQT + 1)
            attn_group(1, 1)
            cc_piece(1, 1)
            yrecv0(1)
            ypiece(0, 0, 0)
            ypiece(0, 0, 1)

            qkv_chunk(NQT + 2)
            attn_group(1, 2)
            cc_piece(1, 2)
            ypiece(0, 1, 0)
            ypiece(0, 1, 1)

            qkv_chunk(NQT + 3)
            attn_group(1, 3)
            cc_piece(1, 3)
            yrecv1(0)
            yrecv1(1)
            yrecv1(2)
            yrecv1(3)
            ypiece_wide(0)
            ypiece_wide(1)

    nc.compile()
    return nc


def _host_inputs(x, Wq, Wk, Wv, Wo):
    x = np.asarray(x, dtype=np.float32)
    Wq, Wk, Wv, Wo = (np.asarray(w, dtype=np.float32) for w in (Wq, Wk, Wv, Wo))
    toks = x.reshape(TOK, D)
    xT = np.ascontiguousarray(toks.T).astype(NPBF16)
    base = 2.0 ** (-8.0 / H)

    cap = np.where(
        np.arange(128)[:, None] <= np.arange(128)[None, :], 3.0e38, -1.0e9
    ).astype(np.float32)
    pos = np.arange(T, dtype=np.float32)
    pos_bf = pos.astype(NPBF16).astype(np.float32)
    kbrow = np.floor(pos / 128.0).astype(NPBF16)  # k-block index, exact
    ones_row = np.ones(T, dtype=NPBF16)
    kaug = np.stack([ones_row, kbrow, kbrow])  # rows 64..66 of K'

    in_maps = []
    for c in range(NC):
        heads = [c, 8 + c]  # steep slot A, shallow slot B
        rows = np.concatenate([np.arange(64 * g, 64 * (g + 1)) for g in heads])
        qaug = np.zeros((HPC * 3, T), dtype=NPBF16)
        kbias = np.zeros((128, HPC), dtype=np.float32)
        for hl, g in enumerate(heads):
            slope = float(base ** (g + 1))
            qaug[3 * hl + 0] = (-8.0 * slope * pos_bf).astype(NPBF16)
            cc = NPBF16(1024.0 * slope)
            dr = NPBF16(1024.0 * slope - float(cc))
            qaug[3 * hl + 1] = cc
            qaug[3 * hl + 2] = dr
            kbias[:, hl] = slope * np.arange(128)
        in_maps.append(
            {
                "xT": xT,
                "wqkv": np.ascontiguousarray(
                    np.concatenate(
                        [Wq[rows, :].T, Wk[rows, :].T, Wv[rows, :].T], axis=1
                    )
                ).astype(NPBF16),
                "wo": None,  # filled below (same for all cores)
                "qaug": qaug,
                "kaug": kaug,
                "kbias": kbias,
                "cap": cap,
            }
        )
    # Wo rows permuted to match the concat order the a2a produces:
    # source core p contributes [head p dims ; head 8+p dims].
    perm = np.concatenate(
        [
            np.concatenate(
                [np.arange(64 * p, 64 * (p + 1)), np.arange(64 * (8 + p), 64 * (9 + p))]
            )
            for p in range(NC)
        ]
    )
    wo_t = np.ascontiguousarray(Wo.T[perm, :]).astype(NPBF16)
    for m in in_maps:
        m["wo"] = wo_t
    return in_maps


def get_compiled():
    global _COMPILED
    if _COMPILED is None:
        _COMPILED = _build()
    return _COMPILED


def run(x, Wq, Wk, Wv, Wo, trace=False, **trace_kwargs):
    nc = get_compiled()
    in_maps = _host_inputs(x, Wq, Wk, Wv, Wo)
    res = bass_utils.run_bass_kernel_spmd(
        nc, in_maps, core_ids=list(range(NC)), trace=trace, **trace_kwargs
    )
    full = np.empty((TOK, D), dtype=np.float32)
    # o row [256b + 128tb + 64h + j] = batch b, query 512*(2tb+h) + 64c + j
    for c in range(NC):
        o = res.results[c]["out"]
        for b in range(B):
            for qt in range(NQT):
                full[T * b + 512 * qt + 64 * c : T * b + 512 * qt + 64 * c + 64, :] = o[
                    256 * b + 64 * qt : 256 * b + 64 * qt + 64
                ]
    return full.reshape(B, T, D), res


def kernel(x, Wq, Wk, Wv, Wo):
    out, _ = run(x, Wq, Wk, Wv, Wo)
    return out
